# revision 1
# baseline (speedup 1.0000x reference)
"""DIoU regression loss on 8 Trainium2 NeuronCores (data-parallel).

loss = sum(1 - clip(diou(pred_i, gt_i), -1, 1)) / (N + 1e-4) over N=4M boxes.

Sharding: each core gets a contiguous slab of R = 128*T*K rows; the last
core's slab is padded with identical unit boxes whose diou == 1, so padded
rows contribute 0 to sum(1 - diou). Each core returns per-partition sums of
clip(diou); the host combines: loss = (N_padded - sum(diou)) / (N + 1e-4).

Math (equivalent to the det3d corner-based reference):
  full extents per box: Ex = w*cos(r) + l*sin(r), Ey = l*cos(r) - w*sin(r),
  Ez = h.  For a dim with centers (cp, cg) and full extents (Ep, Eg):
    inter = relu(min(Ep, Eg, (Ep+Eg)/2 - |cg-cp|))
    outer = relu(max(Ep, Eg, (Ep+Eg)/2 + |cg-cp|))
  (identical to min/max over the reference's corner0/corner2 expressions).
"""

import numpy as np

import concourse.bacc as bacc
import concourse.mybir as mybir
import concourse.tile as tile
from concourse import bass_utils

P = 128          # SBUF partitions
T = 652          # rows per partition per tile
K = 6            # tiles per core
NCORES = 8
RCORE = P * T * K            # 500,736 rows per core
NPAD = RCORE * NCORES        # 4,005,888
NREAL = 4_000_000
D = 9
F32 = mybir.dt.float32
CT = mybir.dt.float32        # compute dtype for elementwise temps
HALF_PI = float(np.pi / 2)

AF = mybir.ActivationFunctionType
OP = mybir.AluOpType

_PAD_ROW = np.array([0, 0, 0, 1, 1, 1, 0, 0, 0], dtype=np.float32)

_CACHE = {}
_TRACE = False
_LAST = None
_DEBUG = False


def _build():
    nc = bacc.Bacc("TRN2", target_bir_lowering=False, debug=False,
                   num_devices=NCORES)
    pred = nc.dram_tensor("pred", [RCORE, D], F32, kind="ExternalInput").ap()
    gt = nc.dram_tensor("gt", [RCORE, D], F32, kind="ExternalInput").ap()
    out = nc.dram_tensor("out", [P, 1], F32, kind="ExternalOutput").ap()
    dbg = {}
    if _DEBUG:
        for nm in ("cs_p", "Ex_p", "diou", "acc"):
            shp = [P, K] if nm == "acc" else [P, T]
            dbg[nm] = nc.dram_tensor(f"dbg_{nm}", shp, F32,
                                     kind="ExternalOutput").ap()

    predv = pred.rearrange("(k p t) d -> k p t d", p=P, t=T)
    gtv = gt.rearrange("(k p t) d -> k p t d", p=P, t=T)

    with tile.TileContext(nc) as tc:
        with (
            tc.tile_pool(name="raw", bufs=2) as raw,
            tc.tile_pool(name="tmp", bufs=1) as tmp,
            tc.tile_pool(name="one", bufs=1) as one,
        ):
            acc = one.tile([P, K], F32, tag="acc", name="acc")
            halfpi = one.tile([P, 1], F32, tag="halfpi", name="halfpi")
            nc.vector.memset(halfpi, HALF_PI)

            for i in range(K):
                praw = raw.tile([P, T, D], F32, tag="praw", name="praw")
                graw = raw.tile([P, T, D], F32, tag="graw", name="graw")
                nc.sync.dma_start(out=praw, in_=predv[i])
                nc.sync.dma_start(out=graw, in_=gtv[i])

                def t(tag):
                    return tmp.tile([P, T], CT, tag=tag, name=tag)

                # --- per-box: extents Ex, Ey and volume ---
                ext = {}   # (box, dim) -> extent tile;  vols[box]
                vols = {}
                for box, rw in (("p", praw), ("g", graw)):
                    x_, y_, z_, w_, l_, h_, r_ = (rw[:, :, c] for c in range(7))
                    sn = t(f"sn_{box}")
                    cs = t(f"cs_{box}")
                    nc.scalar.activation(out=sn, in_=r_, func=AF.Sin)
                    # cos(r) = sin(pi/2 - r); keeps the arg in (0.57, 1.57]
                    # (the ACT Sin spline's domain does not cover r + pi/2).
                    nc.scalar.activation(out=cs, in_=r_, func=AF.Sin,
                                         bias=halfpi, scale=-1.0)
                    t1 = t(f"t1_{box}")   # becomes Ex
                    t4 = t(f"t4_{box}")   # becomes Ey
                    t2 = t("t2")
                    t3 = t("t3")
                    nc.vector.tensor_mul(t1, w_, cs)
                    nc.vector.tensor_mul(t2, l_, sn)
                    nc.vector.tensor_mul(t3, w_, sn)
                    nc.vector.tensor_mul(t4, l_, cs)
                    nc.vector.tensor_add(t1, t1, t2)      # Ex
                    nc.vector.tensor_sub(t4, t4, t3)      # Ey
                    vol = t(f"vol_{box}")
                    nc.gpsimd.tensor_tensor(out=vol, in0=w_, in1=l_, op=OP.mult)
                    nc.gpsimd.tensor_tensor(out=vol, in0=vol, in1=h_, op=OP.mult)
                    ext[(box, 0)] = (x_, t1)
                    ext[(box, 1)] = (y_, t4)
                    ext[(box, 2)] = (z_, h_)
                    vols[box] = vol

                inters = []
                outers2 = []
                deltas2 = []
                for dim in range(3):
                    cp, Ep = ext[("p", dim)]
                    cg, Eg = ext[("g", dim)]
                    delta = t(f"delta_{dim}")    # later squared in place
                    nc.vector.tensor_sub(delta, cg, cp)
                    ad = t("ad")
                    nc.scalar.activation(out=ad, in_=delta, func=AF.Abs)
                    m = t("m")
                    M = t("M")
                    S = t("S")
                    nc.vector.tensor_tensor(out=m, in0=Ep, in1=Eg, op=OP.min)
                    nc.vector.tensor_tensor(out=M, in0=Ep, in1=Eg, op=OP.max)
                    nc.vector.tensor_add(S, Ep, Eg)
                    t1d = t("t1d")
                    t2d = t("t2d")
                    # (S * 0.5) -/+ ad
                    nc.vector.scalar_tensor_tensor(out=t1d, in0=S, scalar=0.5,
                                                   in1=ad, op0=OP.mult,
                                                   op1=OP.subtract)
                    nc.vector.scalar_tensor_tensor(out=t2d, in0=S, scalar=0.5,
                                                   in1=ad, op0=OP.mult,
                                                   op1=OP.add)
                    i0 = t(f"i_{dim}")
                    nc.vector.tensor_tensor(out=i0, in0=m, in1=t1d, op=OP.min)
                    nc.vector.tensor_scalar_max(i0, i0, 0.0)   # inter_d
                    o0 = t("o0")
                    nc.vector.tensor_tensor(out=o0, in0=M, in1=t2d, op=OP.max)
                    nc.vector.tensor_scalar_max(o0, o0, 0.0)
                    o2 = t(f"o2_{dim}")
                    nc.scalar.activation(out=o2, in_=o0, func=AF.Square)
                    nc.scalar.activation(out=delta, in_=delta, func=AF.Square)
                    inters.append(i0)
                    outers2.append(o2)
                    deltas2.append(delta)

                # idiag = dx2+dy2+dz2 (into deltas2[0]); odiag into outers2[0]
                idiag = deltas2[0]
                nc.gpsimd.tensor_tensor(out=idiag, in0=idiag, in1=deltas2[1], op=OP.add)
                nc.gpsimd.tensor_tensor(out=idiag, in0=idiag, in1=deltas2[2], op=OP.add)
                odiag = outers2[0]
                nc.gpsimd.tensor_tensor(out=odiag, in0=odiag, in1=outers2[1], op=OP.add)
                nc.gpsimd.tensor_tensor(out=odiag, in0=odiag, in1=outers2[2], op=OP.add)
                iv = inters[0]
                nc.vector.tensor_mul(iv, iv, inters[1])
                nc.vector.tensor_mul(iv, iv, inters[2])
                un = vols["p"]
                nc.vector.tensor_add(un, un, vols["g"])
                nc.vector.tensor_sub(un, un, iv)
                nc.vector.reciprocal_approx_fast(out=un, in_=un)        # 1/union
                nc.vector.reciprocal_approx_fast(out=odiag, in_=odiag)  # 1/odiag
                nc.vector.tensor_mul(iv, iv, un)          # r1
                nc.vector.tensor_mul(idiag, idiag, odiag)  # r2
                nc.vector.tensor_sub(iv, iv, idiag)       # diou (uncl.)
                # clip to [-1,1] and row-sum into acc[:, i]
                nc.vector.tensor_scalar(out=iv, in0=iv, scalar1=1.0,
                                        scalar2=-1.0, op0=OP.min, op1=OP.max)
                nc.vector.tensor_reduce(acc[:, i:i + 1], iv,
                                        axis=mybir.AxisListType.X, op=OP.add)
                if _DEBUG and i == 0:
                    nc.sync.dma_start(out=dbg["diou"], in_=iv)
                    nc.sync.dma_start(out=dbg["cs_p"], in_=ext[("p", 0)][1])

            if _DEBUG:
                nc.sync.dma_start(out=dbg["acc"], in_=acc)

            red = one.tile([P, 1], F32, tag="red", name="red")
            nc.vector.tensor_reduce(red, acc, axis=mybir.AxisListType.X,
                                    op=OP.add)
            nc.sync.dma_start(out=out, in_=red)

    nc.compile()
    return nc


def kernel(box_pred, box_gt):
    global _LAST
    box_pred = np.asarray(box_pred, dtype=np.float32)
    box_gt = np.asarray(box_gt, dtype=np.float32)
    n = box_pred.shape[0]
    assert n == NREAL, f"kernel hardcoded for N={NREAL}, got {n}"

    if "nc" not in _CACHE:
        _CACHE["nc"] = _build()
    nc = _CACHE["nc"]

    npad = NPAD - NREAL
    pad = np.broadcast_to(_PAD_ROW, (npad, D))
    in_maps = []
    for c in range(NCORES):
        lo, hi = c * RCORE, (c + 1) * RCORE
        if hi <= NREAL:
            p_sl, g_sl = box_pred[lo:hi], box_gt[lo:hi]
        else:
            p_sl = np.concatenate([box_pred[lo:NREAL], pad], axis=0)
            g_sl = np.concatenate([box_gt[lo:NREAL], pad], axis=0)
        in_maps.append({"pred": p_sl, "gt": g_sl})

    kw = dict(trace=True, trace_cores=[0]) if _TRACE else {}
    res = bass_utils.run_bass_kernel_spmd(nc, in_maps,
                                          core_ids=list(range(NCORES)), **kw)
    _LAST = res
    total_diou = sum(
        float(res.results[c]["out"].astype(np.float64).sum())
        for c in range(NCORES)
    )
    loss = (NPAD - total_diou) / (NREAL + 1e-4)
    return np.float32(loss)



# revision 11
# speedup vs baseline: 1.8496x; 1.8496x over previous
"""DIoU regression loss on 8 Trainium2 NeuronCores (data-parallel).

loss = sum(1 - clip(diou(pred_i, gt_i), -1, 1)) / (N + 1e-4) over N=4M boxes.

Sharding: each core gets a contiguous slab of R = 128*T*K rows; the last
core's slab is padded with identical unit boxes whose diou == 1, so padded
rows contribute 0 to sum(1 - diou).

Layout: the host packs the 7 used columns of both boxes into a plane-major
bf16 array [14, RCORE] per core, so every on-chip operand is a unit-stride
[P, T] (or [P, 2, T]) bf16 slice -> DVE runs in its 2x/4x packed modes and
DMA traffic is 14 MB/core instead of 36 MB.

Math (equivalent to the det3d corner-based reference): with full extents
  Ex = w*cos(r) + l*sin(r), Ey = l*cos(r) - w*sin(r), Ez = h
and per dim u = Ep+Eg, v = Ep-Eg, dc = cg-cp, g = max(|v|, |2dc|):
  2*inter_d = relu(u - g),   2*outer_d = u + g  (relu'd for y)
so with I = prod(relu(u-g)) = 8*inter_vol, U8 = 8*(volp+volg) - I = 8*union,
O4 = sum((u+g)^2) = 4*outer_diag, D = sum(dc^2):
  -diou = (4*D*U8 - I*O4) / (U8*O4)
Each core emits per-partition per-tile sums of clip(-diou, -1, 1); the host
combines: loss = (NPAD + total_negdiou) / (N + 1e-4).
"""

import numpy as np
import ml_dtypes

import concourse.bacc as bacc
import concourse.mybir as mybir
import concourse.tile as tile
from concourse import bass_utils

P = 128          # SBUF partitions
T = 1304         # rows per partition per tile
K = 3            # tiles per core
NCORES = 8
RCORE = P * T * K            # 500,736 rows per core
NPAD = RCORE * NCORES        # 4,005,888
NREAL = 4_000_000
C = 14                       # planes
BF16 = mybir.dt.bfloat16
F32 = mybir.dt.float32
HALF_PI = float(np.pi / 2)

AF = mybir.ActivationFunctionType
OP = mybir.AluOpType

# plane order: w_p w_g l_p l_g h_p h_g r_p r_g x_p y_p x_g y_g z_p z_g
_PLANE_SRC = [(3, 0), (3, 1), (4, 0), (4, 1), (5, 0), (5, 1), (6, 0), (6, 1),
              (0, 0), (1, 0), (0, 1), (1, 1), (2, 0), (2, 1)]
_PLANE_PAD = np.array([1, 1, 1, 1, 1, 1, 0, 0, 0, 0, 0, 0, 0, 0],
                      dtype=np.float32)

_CACHE = {}
_TRACE = False
_LAST = None
_DEBUG = False


def _build():
    nc = bacc.Bacc("TRN2", target_bir_lowering=False, debug=False,
                   num_devices=NCORES)
    ab = nc.dram_tensor("ab", [C, RCORE], BF16, kind="ExternalInput").ap()
    out = nc.dram_tensor("out", [P, K], F32, kind="ExternalOutput").ap()
    dbg = {}
    if _DEBUG:
        for nm in ("w2", "sn2", "Ep2", "g_xy", "ti_xy", "to_xy"):
            dbg[nm] = nc.dram_tensor(f"dbg_{nm}", [P, 2, T], BF16,
                                     kind="ExternalOutput").ap()
        for nm in ("I", "idiag", "O4", "U8", "negnum"):
            dbg[nm] = nc.dram_tensor(f"dbg_{nm}", [P, 1, T], BF16,
                                     kind="ExternalOutput").ap()
        for nm in ("rcp", "negd", "dump"):
            dbg[nm] = nc.dram_tensor(f"dbg_{nm}", [P, 1, T], F32,
                                     kind="ExternalOutput").ap()

    # [k][P, C, T]: plane c of tile k, partition p starts at
    # c*RCORE + k*P*T + p*T
    abv = ab.rearrange("c (k p t) -> k p c t", p=P, t=T)

    with tile.TileContext(nc) as tc:
        with (
            tc.tile_pool(name="raw", bufs=2) as rawp,
            tc.tile_pool(name="tmp", bufs=1) as tmp,
            tc.tile_pool(name="one", bufs=1) as one,
        ):
            acc = one.tile([P, K], F32, tag="acc", name="acc")
            halfpi = one.tile([P, 1], F32, tag="halfpi", name="halfpi")
            nc.vector.memset(halfpi, HALF_PI)

            # physical buffer tags are reused once the previous logical
            # tenant's last reader is done (WAR handled by the tile dep
            # tracker); GPSIMD-owned tiles (wl2/vol2/S) are never aliased
            # with VEC-hot tiles so VEC never stalls on GP.
            _ALIAS = {
                "sn2": "A0", "cs2": "A1", "wl2": "A2", "vol2": "A3",
                "wc2": "A4", "ls2": "A5", "ws2": "A6", "lc2": "A7",
                "Ep2": "A8", "Eg2": "A9",
                "u_xy": "A4", "v_xy": "A5", "dc_xy": "A6", "g_xy": "A7",
                "ti_xy": "A8", "to_xy": "A9", "d2_xy": "A0", "o2_xy": "A1",
                "a2_xy": "A8", "a2_z": "B5",
                "S": "B0", "u_z": "B1", "v_z": "B2", "dc_z": "B3",
                "g_z": "B4", "ti_z": "B5", "to_z": "B6",
                "d2_z": "B2", "o2_z": "B4", "I": "B1", "idiag": "B3",
                "O4": "B6", "U8": "B5", "IO": "B2", "DU": "B4",
                "negnum": "B7",
                "UO": "X0", "rcp": "X1", "negd": "X2", "dump": "X0",
            }

            def t2(tag):
                t = _ALIAS[tag]
                return tmp.tile([P, 2, T], BF16, tag=t, name=tag)

            def t1(tag):
                t = _ALIAS[tag]
                return tmp.tile([P, 1, T], BF16, tag=t, name=tag)

            def f1(tag):
                t = _ALIAS[tag]
                return tmp.tile([P, 1, T], F32, tag=t, name=tag)

            for k in range(K):
                raw = rawp.tile([P, C, T], BF16, tag="raw", name="raw")
                nc.sync.dma_start(out=raw, in_=abv[k])

                w2 = raw[:, 0:2]
                l2 = raw[:, 2:4]
                h2 = raw[:, 4:6]
                r2 = raw[:, 6:8]
                cp_xy = raw[:, 8:10]
                cg_xy = raw[:, 10:12]
                zp = raw[:, 12:13]
                zg = raw[:, 13:14]

                # --- trig (ACT) ---
                sn2 = t2("sn2")
                cs2 = t2("cs2")
                nc.scalar.activation(out=sn2, in_=r2, func=AF.Sin)
                # cos(r) = sin(pi/2 - r); keeps the arg inside the Sin
                # spline's domain.
                nc.scalar.activation(out=cs2, in_=r2, func=AF.Sin,
                                     bias=halfpi, scale=-1.0)

                # --- volumes (GPSIMD) ---
                wl2 = t2("wl2")
                vol2 = t2("vol2")
                S = t1("S")
                nc.gpsimd.tensor_tensor(out=wl2, in0=w2, in1=l2, op=OP.mult)
                nc.gpsimd.tensor_tensor(out=vol2, in0=wl2, in1=h2, op=OP.mult)
                nc.gpsimd.tensor_tensor(out=S, in0=vol2[:, 0:1],
                                        in1=vol2[:, 1:2], op=OP.add)

                # --- rotated extents (VEC) ---
                wc2 = t2("wc2")
                ls2 = t2("ls2")
                ws2 = t2("ws2")
                lc2 = t2("lc2")
                nc.vector.tensor_mul(wc2, w2, cs2)
                nc.vector.tensor_mul(ls2, l2, sn2)
                nc.vector.tensor_mul(ws2, w2, sn2)
                nc.vector.tensor_mul(lc2, l2, cs2)
                Ep2 = t2("Ep2")   # (Ex_p, Ey_p)
                Eg2 = t2("Eg2")
                nc.vector.tensor_add(Ep2[:, 0:1], wc2[:, 0:1], ls2[:, 0:1])
                nc.vector.tensor_sub(Ep2[:, 1:2], lc2[:, 0:1], ws2[:, 0:1])
                nc.vector.tensor_add(Eg2[:, 0:1], wc2[:, 1:2], ls2[:, 1:2])
                nc.vector.tensor_sub(Eg2[:, 1:2], lc2[:, 1:2], ws2[:, 1:2])

                # --- per-dim u, v, dc ---
                u_xy = t2("u_xy")
                v_xy = t2("v_xy")
                dc_xy = t2("dc_xy")
                nc.vector.tensor_add(u_xy, Ep2, Eg2)
                nc.vector.tensor_sub(v_xy, Ep2, Eg2)
                nc.vector.tensor_sub(dc_xy, cg_xy, cp_xy)
                u_z = t1("u_z")
                v_z = t1("v_z")
                dc_z = t1("dc_z")
                nc.vector.tensor_add(u_z, h2[:, 0:1], h2[:, 1:2])
                nc.vector.tensor_sub(v_z, h2[:, 0:1], h2[:, 1:2])
                nc.vector.tensor_sub(dc_z, zg, zp)

                # --- g = max(|2dc|, |v|) = max(max(v, |2dc|), -v) ---
                a2_xy = t2("a2_xy")
                a2_z = t1("a2_z")
                nc.scalar.activation(out=a2_xy, in_=dc_xy, func=AF.Abs,
                                     scale=2.0)
                nc.scalar.activation(out=a2_z, in_=dc_z, func=AF.Abs,
                                     scale=2.0)
                g_xy = t2("g_xy")
                g_z = t1("g_z")
                nc.vector.tensor_tensor(out=g_xy, in0=v_xy, in1=a2_xy,
                                        op=OP.max)
                nc.vector.scalar_tensor_tensor(out=g_xy, in0=v_xy, scalar=-1.0,
                                               in1=g_xy, op0=OP.mult,
                                               op1=OP.max)
                nc.vector.tensor_tensor(out=g_z, in0=v_z, in1=a2_z, op=OP.max)
                nc.vector.scalar_tensor_tensor(out=g_z, in0=v_z, scalar=-1.0,
                                               in1=g_z, op0=OP.mult,
                                               op1=OP.max)

                # --- 2*inter = relu(u-g); 2*outer = u+g ---
                ti_xy = t2("ti_xy")
                to_xy = t2("to_xy")
                ti_z = t1("ti_z")
                to_z = t1("to_z")
                nc.vector.tensor_sub(ti_xy, u_xy, g_xy)
                nc.vector.tensor_add(to_xy, u_xy, g_xy)
                nc.vector.tensor_sub(ti_z, u_z, g_z)
                nc.vector.tensor_add(to_z, u_z, g_z)
                nc.vector.tensor_scalar_max(ti_xy, ti_xy, 0.0)
                nc.vector.tensor_scalar_max(ti_z, ti_z, 0.0)
                # only outer_y can be negative
                nc.vector.tensor_scalar_max(to_xy[:, 1:2], to_xy[:, 1:2], 0.0)

                # --- squares (ACT) ---
                d2_xy = t2("d2_xy")
                d2_z = t1("d2_z")
                o2_xy = t2("o2_xy")
                o2_z = t1("o2_z")
                nc.scalar.activation(out=d2_xy, in_=dc_xy, func=AF.Square)
                nc.scalar.activation(out=d2_z, in_=dc_z, func=AF.Square)
                nc.scalar.activation(out=o2_xy, in_=to_xy, func=AF.Square)
                nc.scalar.activation(out=o2_z, in_=to_z, func=AF.Square)

                # --- reduce over dims ---
                I = t1("I")
                nc.vector.tensor_mul(I, ti_xy[:, 0:1], ti_xy[:, 1:2])
                nc.vector.tensor_mul(I, I, ti_z)
                idiag = t1("idiag")
                nc.vector.tensor_add(idiag, d2_xy[:, 0:1], d2_xy[:, 1:2])
                nc.vector.tensor_add(idiag, idiag, d2_z)
                O4 = t1("O4")
                nc.vector.tensor_add(O4, o2_xy[:, 0:1], o2_xy[:, 1:2])
                nc.vector.tensor_add(O4, O4, o2_z)

                # --- -diou = (4*idiag*U8 - I*O4) / (U8*O4) ---
                U8 = t1("U8")
                nc.vector.scalar_tensor_tensor(out=U8, in0=S, scalar=8.0,
                                               in1=I, op0=OP.mult,
                                               op1=OP.subtract)
                UO = f1("UO")
                nc.vector.tensor_mul(UO, U8, O4)
                rcp = f1("rcp")
                nc.vector.reciprocal_approx_fast(out=rcp, in_=UO)
                IO = t1("IO")
                DU = t1("DU")
                nc.vector.tensor_mul(IO, I, O4)
                nc.vector.tensor_mul(DU, idiag, U8)
                negnum = t1("negnum")
                nc.vector.scalar_tensor_tensor(out=negnum, in0=DU, scalar=4.0,
                                               in1=IO, op0=OP.mult,
                                               op1=OP.subtract)
                negd = f1("negd")
                nc.vector.tensor_mul(negd, negnum, rcp)
                # clip to [-1, 1] and row-sum into acc[:, k] in one op
                dump = f1("dump")
                nc.vector.tensor_scalar(out=dump, in0=negd, scalar1=1.0,
                                        scalar2=-1.0, op0=OP.min, op1=OP.max)
                nc.vector.tensor_reduce(acc[:, k:k + 1], dump[:, 0],
                                        axis=mybir.AxisListType.X, op=OP.add)

                if _DEBUG and k == 0:
                    for nm, tl in (("w2", raw[:, 0:2]), ("sn2", sn2),
                                   ("Ep2", Ep2), ("g_xy", g_xy),
                                   ("ti_xy", ti_xy), ("to_xy", to_xy),
                                   ("I", I), ("idiag", idiag), ("O4", O4),
                                   ("U8", U8), ("negnum", negnum),
                                   ("rcp", rcp), ("negd", negd),
                                   ("dump", dump)):
                        nc.sync.dma_start(out=dbg[nm], in_=tl)

            nc.sync.dma_start(out=out, in_=acc)

    nc.compile()
    return nc


def _pack_planes(box_pred, box_gt):
    """Full [C, NPAD] bf16 plane-major array."""
    planes = np.empty((C, NPAD), dtype=ml_dtypes.bfloat16)
    src = (box_pred, box_gt)
    for i, (col, which) in enumerate(_PLANE_SRC):
        planes[i, :NREAL] = src[which][:, col].astype(ml_dtypes.bfloat16)
        planes[i, NREAL:] = _PLANE_PAD[i]
    return planes


def kernel(box_pred, box_gt):
    global _LAST
    box_pred = np.asarray(box_pred, dtype=np.float32)
    box_gt = np.asarray(box_gt, dtype=np.float32)
    n = box_pred.shape[0]
    assert n == NREAL, f"kernel hardcoded for N={NREAL}, got {n}"

    if "nc" not in _CACHE:
        _CACHE["nc"] = _build()
    nc = _CACHE["nc"]

    planes = _pack_planes(box_pred, box_gt)
    in_maps = []
    for c in range(NCORES):
        lo, hi = c * RCORE, (c + 1) * RCORE
        in_maps.append({"ab": np.ascontiguousarray(planes[:, lo:hi])})

    kw = dict(trace=True, trace_cores=[0]) if _TRACE else {}
    res = bass_utils.run_bass_kernel_spmd(nc, in_maps,
                                          core_ids=list(range(NCORES)), **kw)
    _LAST = res
    total_neg = sum(
        float(res.results[c]["out"].astype(np.float64).sum())
        for c in range(NCORES)
    )
    loss = (NPAD + total_neg) / (NREAL + 1e-4)
    return np.float32(loss)
